# revision 62
# baseline (speedup 1.0000x reference)
"""1-D Winograd F(4,3) along W for the per-sample adaptive conv.

Host prep: pad, de-interleave into stride-4 phase planes, and apply the
(scaled) B^T input transform per group of 4 output columns:
  d = x_pad[4tx .. 4tx+5]
  V0 = d0 - 1.25 d2 + 0.25 d4          (= row0(B^T)/4)
  V1 = (d1+d2) - 0.25 (d3+d4)          (= -row1/4)
  V2 = (d1-d2) - 0.25 (d3-d4)          (= row2/4)
  V3 = (d3-d1) + 0.5 (d4-d2)           (= row3/2)
  V4 = -(d3-d1) + 0.5 (d4-d2)          (= row4/2)
  V5 = d1 - 1.25 d3 + 0.25 d5          (= row5/4)
plus the weight G-transform (inverse row scales folded in).

Device: the full contraction
  m_jx = sum_{cic,ky} Gw[jx][co,ci,ky] * V_jx[ci, y+ky, tx]   (TensorE)
and the A^T output transform
  out[4tx+0] = m0+m1+m2+m3+m4
  out[4tx+1] = (m1-m2) + 2(m3-m4)
  out[4tx+2] = (m1+m2) + 4(m3+m4)
  out[4tx+3] = (m1-m2) + 8(m3-m4) + m5
with m staged PSUM->SBUF as bf16 by ScalarE and the combine on DVE
(tensor_tensor 2x + tensor_scalar 4x perf modes). Output ships bf16 and
is widened to fp32 on host.

MACs: 6 jx x 6 (cic,ky) x 512 -> 576 matmuls/core vs 768 for F(2,3).
The H-pad rows (global rows 0 and 65) are all-zero, so the one ky per
group that touches a pad row is trimmed to 31 rows (N=496), shaving
~1.3us off the PE-bound matmul stream (measured ~2-3us incl. knock-on
scheduling effects).
"""

import numpy as np
import ml_dtypes

B, T, CIN, COUT, H, W = 8, 4, 256, 256, 64, 64
KH, KW = 3, 3
NCORES = 8
CH = 2
NJX = 6             # winograd positions per tile
NTX = W // 4        # 16 tiles of 4 output cols per row
HP = H + 2          # 66 padded rows
YB_ROWS = 32        # output rows per psum tile -> N = 32*16 = 512
NYB = H // YB_ROWS  # 2

XROW = NJX * NTX    # 96 V values per padded row (stored plane-major)
NW = CH * NJX * CH * KH  # 72 weight tiles

_cache = {}
LAST_EXEC_TIME_NS = None
LAST_PROFILE = None


def _build():
    import concourse.mybir as mybir
    import concourse.tile as tile
    from concourse import bacc

    ALU = mybir.AluOpType

    nc = bacc.Bacc(
        "TRN2",
        target_bir_lowering=False,
        debug=False,
        enable_asserts=False,
        num_devices=NCORES,
    )
    x_d = nc.dram_tensor(
        "x", [T, CH, 128, HP * XROW], mybir.dt.bfloat16, kind="ExternalInput"
    ).ap()
    w_d = nc.dram_tensor(
        "w", [128, NW * 128], mybir.dt.bfloat16, kind="ExternalInput"
    ).ap()
    o_d = nc.dram_tensor(
        "out", [T, CH, 128, H * W], mybir.dt.bfloat16, kind="ExternalOutput"
    ).ap()

    ROW_BLOCKS = [(0, 34), (34, 50), (50, 66)]

    def widx(coc, jx, cic, ky):
        return ((coc * NJX + jx) * CH + cic) * KH + ky

    with tile.TileContext(nc) as tc:
        with (
            tc.tile_pool(name="persist", bufs=1) as persist,
            tc.tile_pool(name="xv", bufs=2) as xv_pool,
            tc.tile_pool(name="psum", bufs=8, space="PSUM") as psum_pool,
            tc.tile_pool(name="obuf", bufs=2) as out_pool,
        ):
            w_sb = persist.tile([128, NW * 128], mybir.dt.bfloat16, tag="w")

            # V tiles (host-transformed input), double-buffered across images
            x_sb = {}
            for t in range(T):
                for c in range(CH):
                    x_sb[(t, c)] = xv_pool.tile(
                        [128, HP * XROW],
                        mybir.dt.bfloat16,
                        name=f"x{t}{c}",
                        tag=f"x{c}",
                        bufs=2,
                    )

            # PE warmup bridge: DVE memset (GpSimd's queue launches ~5us
            # late) + tiny N=64 matmuls from ~7.3us to ~14us. This (a)
            # completes the pstate ramp (~5.3us of continuous PE busy)
            # BEFORE the first real chain, and (b) lets ~1.5MB of input
            # accumulate so the stream can then run at full clock without
            # the starvation gaps that would otherwise reset the ramp.
            warm_x = persist.tile([128, 192], mybir.dt.bfloat16, name="warm", tag="warm")
            warm_ps = psum_pool.tile(
                [128, 512], mybir.dt.float32, name="wps", tag="m", bufs=8
            )
            nc.vector.memset(warm_x[:], 0.0)
            for _ in range(160):
                nc.tensor.matmul(
                    warm_ps[:, :64], warm_x[:, :128], warm_x[:, 128:192],
                    start=True, stop=True,
                )

            # DMA order: first-needed slivers first; image-0 chunk 0 on the
            # sync ring, weights + chunk 1 on the scalar ring so the critical
            # tiles land in parallel
            # V is plane-major: [jx, HP, NTX] — matmul rhs slices are fully
            # contiguous runs, which the PE fetches at full rate (16-element
            # runs measured ~28% slower on HW)
            def xv3(t, c):
                return x_sb[(t, c)][:].rearrange(
                    "p (j h w) -> p j (h w)", j=NJX, w=NTX
                )

            def dma_x_block(t, c, blk, engine=None):
                r0, r1 = ROW_BLOCKS[blk]
                eng = engine or nc.sync
                eng.dma_start(
                    xv3(t, c)[:, :, r0 * NTX : r1 * NTX],
                    x_d[t, c, :].rearrange("p (j r) -> p j r", j=NJX)[
                        :, :, r0 * NTX : r1 * NTX
                    ],
                )

            def dma_x1_block(blk):
                dma_x_block(0, 1, blk, engine=nc.scalar)

            # image-0 weights + inputs are 5.6MB over two HWDGE rings —
            # balance ~2.8MB/ring and order by first-use time
            # ACT (scalar ring) issues only the first four DMAs — anything
            # later would interleave with the PSUM-drain copies and stall the
            # PE on bank frees; the sync ring carries the rest by need time
            nc.scalar.dma_start(w_sb[:, : 6 * 128], w_d[:, : 6 * 128])
            dma_x_block(0, 0, 0)
            dma_x1_block(0)
            nc.scalar.dma_start(w_sb[:, 6 * 128 : 18 * 128], w_d[:, 6 * 128 : 18 * 128])
            nc.scalar.dma_start(w_sb[:, 18 * 128 : 36 * 128], w_d[:, 18 * 128 : 36 * 128])
            dma_x_block(0, 0, 1)
            dma_x_block(0, 0, 2)
            dma_x_block(0, 1, 1, engine=nc.sync)
            dma_x_block(0, 1, 2, engine=nc.sync)
            nc.sync.dma_start(w_sb[:, 36 * 128 : 54 * 128], w_d[:, 36 * 128 : 54 * 128])
            nc.sync.dma_start(w_sb[:, 54 * 128 :], w_d[:, 54 * 128 :])
            for t in range(1, T):
                for blk in range(3):
                    for c in range(CH):
                        dma_x_block(t, c, blk)

            for t in range(T):
                v3 = {
                    c: x_sb[(t, c)][:].rearrange(
                        "p (j h w) -> p j h w", j=NJX, w=NTX
                    )
                    for c in range(CH)
                }
                # coc-outer order: image 0's first two groups reuse weight
                # tiles 0..35, pushing the coc1-weight deadline to ~24us
                group_order = [(coc, yb) for coc in range(CH) for yb in range(NYB)]
                for coc, yb in group_order:
                    y0 = yb * YB_ROWS
                    last = t == T - 1 and coc == CH - 1 and yb == NYB - 1
                    # final group: run the m0 chain last (split in two) so
                    # only a short N=256 chain gates the trailing output
                    jx_order = [1, 2, 3, 4, 5, 0] if last else range(NJX)
                    m = [None] * NJX
                    m0ab = []

                    def mm_chain(mp, jx, rr0, nrows):
                        # the H-pad rows are all-zero, so the ky touching
                        # one (ky=0 at global row 0, ky=2 at row 65) is
                        # trimmed to nrows-1 rows; a full-width ky runs
                        # first so start=True covers the whole psum tile
                        if rr0 == 0 and nrows == YB_ROWS:
                            ky_order, trim = (1, 0, 2), 0
                        elif rr0 + nrows + 2 == HP and nrows == YB_ROWS:
                            ky_order, trim = (0, 1, 2), 2
                        else:
                            ky_order, trim = (0, 1, 2), None
                        k = 0
                        for cic in range(CH):
                            for ky in ky_order:
                                idx = widx(coc, jx, cic, ky)
                                if ky == trim and ky == 0:
                                    rhs = v3[cic][
                                        :, jx, rr0 + 1 : rr0 + nrows, :
                                    ]
                                    out = mp[:, NTX : nrows * NTX]
                                elif ky == trim:
                                    rhs = v3[cic][
                                        :, jx,
                                        rr0 + ky : rr0 + ky + nrows - 1, :,
                                    ]
                                    out = mp[:, : (nrows - 1) * NTX]
                                else:
                                    rhs = v3[cic][
                                        :, jx, rr0 + ky : rr0 + ky + nrows, :
                                    ]
                                    out = mp[:]
                                nc.tensor.matmul(
                                    out,
                                    w_sb[:, idx * 128 : (idx + 1) * 128],
                                    rhs,
                                    start=(k == 0),
                                    stop=(k == CH * KH - 1),
                                )
                                k += 1

                    for jx in jx_order:
                        if last and jx == 0:
                            half = YB_ROWS // 2
                            for h in range(2):
                                mp = psum_pool.tile(
                                    [128, half * NTX],
                                    mybir.dt.float32,
                                    name=f"m0{h}",
                                    tag="m",
                                    bufs=8,
                                )
                                mm_chain(mp, 0, y0 + h * half, half)
                                m0ab.append(mp)
                            continue
                        mp = psum_pool.tile(
                            [128, YB_ROWS * NTX],
                            mybir.dt.float32,
                            name=f"m{jx}",
                            tag="m",
                            bufs=8,
                        )
                        mm_chain(mp, jx, y0, YB_ROWS)
                        m[jx] = mp

                    NEL = YB_ROWS * NTX
                    cst = [
                        out_pool.tile(
                            [128, NEL],
                            mybir.dt.bfloat16,
                            name=f"c{j}",
                            tag=f"c{j}",
                            bufs=2,
                        )
                        for j in range(NJX)
                    ]
                    for j in jx_order:
                        if last and j in (0, 5):
                            continue  # o0/o3 read m0/m5 straight from PSUM
                        nc.scalar.copy(cst[j][:], m[j][:])
                    s = out_pool.tile(
                        [128, NEL], mybir.dt.bfloat16, name="s", tag="s", bufs=2
                    )
                    dd = out_pool.tile(
                        [128, NEL], mybir.dt.bfloat16, name="dd", tag="dd", bufs=2
                    )
                    a = out_pool.tile(
                        [128, NEL], mybir.dt.bfloat16, name="a", tag="a", bufs=2
                    )
                    bb = out_pool.tile(
                        [128, NEL], mybir.dt.bfloat16, name="bb", tag="bb", bufs=2
                    )
                    sc = out_pool.tile(
                        [128, NEL], mybir.dt.bfloat16, name="sc", tag="sc", bufs=2
                    )
                    ob = out_pool.tile(
                        [128, 4 * NEL], mybir.dt.bfloat16, name="ob", tag="ob",
                        bufs=2,
                    )
                    o = [ob[:, j * NEL : (j + 1) * NEL] for j in range(4)]

                    def scaled_add(out, src, k, addend):
                        # (src * k) + addend via ts (4x) + tt (2x) — both
                        # faster DVE paths than the 1x scalar_tensor_tensor
                        nc.vector.tensor_scalar(
                            sc[:], src, k, 0.0, op0=ALU.mult, op1=ALU.add
                        )
                        nc.vector.tensor_add(out, addend, sc[:])

                    base = yb * 4 * NEL
                    if not last:
                        nc.vector.tensor_add(s[:], cst[1][:], cst[2][:])
                        nc.vector.tensor_sub(dd[:], cst[1][:], cst[2][:])
                        nc.vector.tensor_add(a[:], cst[3][:], cst[4][:])
                        nc.vector.tensor_sub(bb[:], cst[3][:], cst[4][:])
                        nc.vector.tensor_add(o[0], cst[0][:], s[:])
                        nc.vector.tensor_add(o[0], o[0], a[:])
                        scaled_add(o[1], bb[:], 2.0, dd[:])
                        scaled_add(o[2], a[:], 4.0, s[:])
                        scaled_add(o[3], bb[:], 8.0, dd[:])
                        nc.vector.tensor_add(o[3], o[3], cst[5][:])
                        nc.gpsimd.dma_start(
                            o_d[t, coc, :, base : base + 4 * NEL], ob[:]
                        )
                    else:
                        # final tile: the DVE sequence is ordered by data
                        # availability — s/dd/o1 unlock at cst2 (T-3.2us),
                        # the cst4-gated ops follow, the o2 pair runs on the
                        # otherwise-idle GpSimd, and o3/o0 read m5/m0
                        # straight from PSUM so only one short DVE op + a
                        # sliver DMA trails each of the last chain pieces
                        nc.vector.tensor_add(s[:], cst[1][:], cst[2][:])
                        nc.vector.tensor_sub(dd[:], cst[1][:], cst[2][:])
                        nc.vector.tensor_add(a[:], cst[3][:], cst[4][:])
                        nc.vector.tensor_sub(bb[:], cst[3][:], cst[4][:])
                        scaled_add(o[1], bb[:], 2.0, dd[:])
                        nc.scalar.dma_start(
                            o_d[t, coc, :, base + NEL : base + 2 * NEL],
                            ob[:, NEL : 2 * NEL],
                        )
                        o0p = cst[0]
                        nc.vector.tensor_add(o0p[:], s[:], a[:])
                        sc2 = cst[5]
                        nc.gpsimd.tensor_scalar(
                            sc2[:], a[:], 4.0, 0.0, op0=ALU.mult, op1=ALU.add
                        )
                        nc.gpsimd.tensor_add(o[2], s[:], sc2[:])
                        nc.sync.dma_start(
                            o_d[t, coc, :, base + 2 * NEL : base + 3 * NEL],
                            ob[:, 2 * NEL : 3 * NEL],
                        )
                        scaled_add(o[3], bb[:], 8.0, dd[:])
                        nc.vector.tensor_add(o[3], o[3], m[5][:])
                        nc.scalar.dma_start(
                            o_d[t, coc, :, base + 3 * NEL : base + 4 * NEL],
                            ob[:, 3 * NEL :],
                        )
                        # o0 halves read m0a/m0b straight from PSUM; the final
                        # slivers ship on the otherwise-idle scalar and sync
                        # rings to dodge the pool issue queue
                        HNEL = NEL // 2
                        nc.vector.tensor_add(
                            o[0][:, :HNEL], m0ab[0][:], o0p[:, :HNEL]
                        )
                        nc.scalar.dma_start(
                            o_d[t, coc, :, base : base + HNEL], o[0][:, :HNEL]
                        )
                        nc.vector.tensor_add(
                            o[0][:, HNEL:], m0ab[1][:], o0p[:, HNEL:]
                        )
                        nc.sync.dma_start(
                            o_d[t, coc, :, base + HNEL : base + NEL],
                            o[0][:, HNEL:],
                        )

    nc.compile()
    return nc


_GP = None


def _gprime():
    global _GP
    if _GP is None:
        G = np.array(
            [
                [1 / 4, 0, 0],
                [-1 / 6, -1 / 6, -1 / 6],
                [-1 / 6, 1 / 6, -1 / 6],
                [1 / 24, 1 / 12, 1 / 6],
                [1 / 24, -1 / 12, 1 / 6],
                [0, 0, 1],
            ],
            dtype=np.float64,
        )
        S = np.diag([4.0, -4.0, 4.0, 2.0, 2.0, 4.0])
        _GP = (S @ G).astype(np.float32)
    return _GP


# scaled B^T rows (the inverse scales are folded into the weights)
_BTS = np.array(
    [
        [1, 0, -1.25, 0, 0.25, 0],
        [0, 1, 1, -0.25, -0.25, 0],
        [0, 1, -1, -0.25, 0.25, 0],
        [0, -1, -0.5, 1, 0.5, 0],
        [0, 1, -0.5, -1, 0.5, 0],
        [0, 1, 0, -1.25, 0, 0.25],
    ],
    dtype=np.float32,
)


def _prep_inputs(inputs, ada_weight):
    bf16 = ml_dtypes.bfloat16
    Gp = _gprime()
    # column gather index: d[..., r, k, tx] = xpad[..., r, 4tx+k]
    cols = 4 * np.arange(NTX)[None, :] + np.arange(NJX)[:, None]  # [k, tx]
    in_maps = []
    for b in range(B):
        xb = inputs[b * T : (b + 1) * T].reshape(T, CH, 128, H, W).astype(bf16)
        xp = np.zeros((T, CH, 128, HP, W + 2), dtype=bf16)
        xp[..., 1 : H + 1, 1 : W + 1] = xb
        d = xp[..., cols].astype(np.float32)  # [T, CH, 128, HP, NJX(k), NTX]
        # winograd input transform V_j = BTS[j] . d  (host side), plane-major
        xd = np.einsum("jk,...rkx->...jrx", _BTS, d).astype(bf16)

        wb = ada_weight[b].astype(np.float32)  # [co, ci, ky, kx]
        g = np.einsum("jk,oiyk->joiy", Gp, wb)  # [jx, co, ci, ky]
        gt = g.reshape(NJX, CH, 128, CH, 128, KH)  # jx coc co cic ci ky
        wprep = gt.transpose(4, 1, 0, 3, 5, 2)  # ci coc jx cic ky co
        wprep = np.ascontiguousarray(wprep.astype(bf16)).reshape(128, NW * 128)
        in_maps.append({"x": xd.reshape(T, CH, 128, HP * XROW), "w": wprep})
    return in_maps


def _unpack_out(res):
    # [T, CH, 128, NYB, 4 j, 32 y, 16 tx] -> [T, C, H, W]
    arr = np.asarray(res, dtype=np.float32).reshape(T, CH, 128, NYB, 4, YB_ROWS, NTX)
    a = arr.transpose(0, 1, 2, 3, 5, 6, 4)  # t ch co yb y tx j
    return a.reshape(T, COUT, H, W)


def _setup_profiling():
    import sys
    import types

    try:
        from antenv.axon_hooks import get_axon_ntff_profile_hook  # noqa: F401

        return
    except ImportError:
        pass
    import antenv
    from trn_agent_boot.trn_boot import _ntff_profile_via_ctypes

    hook = _ntff_profile_via_ctypes("/opt/axon/libaxon_pjrt.so")
    m = types.ModuleType("antenv.axon_hooks")
    m.get_axon_ntff_profile_hook = lambda: hook
    m.set_axon_ntff_profile_hook = lambda h: None
    sys.modules["antenv.axon_hooks"] = m
    antenv.axon_hooks = m

    from concourse import bass_utils

    bass_utils.upload_artifacts = lambda tmpdir: f"file://{tmpdir}"


def kernel(inputs, ada_weight, profile=False, trace_kwargs=None):
    global LAST_EXEC_TIME_NS, LAST_PROFILE
    from concourse.bass_utils import run_bass_kernel_spmd

    if profile:
        _setup_profiling()
    if "nc" not in _cache:
        _cache["nc"] = _build()
    nc = _cache["nc"]

    in_maps = _prep_inputs(np.asarray(inputs), np.asarray(ada_weight))

    kwargs = {}
    if profile:
        kwargs["trace"] = True
        if trace_kwargs:
            kwargs.update(trace_kwargs)
    res = run_bass_kernel_spmd(nc, in_maps, core_ids=list(range(NCORES)), **kwargs)
    if profile:
        LAST_EXEC_TIME_NS = res.exec_time_ns
        LAST_PROFILE = res

    out = np.stack([_unpack_out(res.results[b]["out"]) for b in range(B)])
    return np.ascontiguousarray(out.reshape(B * T, COUT, H, W).astype(np.float32))



# revision 63
# speedup vs baseline: 1.0108x; 1.0108x over previous
"""1-D Winograd F(4,3) along W for the per-sample adaptive conv.

Host prep: pad, de-interleave into stride-4 phase planes, and apply the
(scaled) B^T input transform per group of 4 output columns:
  d = x_pad[4tx .. 4tx+5]
  V0 = d0 - 1.25 d2 + 0.25 d4          (= row0(B^T)/4)
  V1 = (d1+d2) - 0.25 (d3+d4)          (= -row1/4)
  V2 = (d1-d2) - 0.25 (d3-d4)          (= row2/4)
  V3 = (d3-d1) + 0.5 (d4-d2)           (= row3/2)
  V4 = -(d3-d1) + 0.5 (d4-d2)          (= row4/2)
  V5 = d1 - 1.25 d3 + 0.25 d5          (= row5/4)
plus the weight G-transform (inverse row scales folded in).

Device: the full contraction
  m_jx = sum_{cic,ky} Gw[jx][co,ci,ky] * V_jx[ci, y+ky, tx]   (TensorE)
and the A^T output transform
  out[4tx+0] = m0+m1+m2+m3+m4
  out[4tx+1] = (m1-m2) + 2(m3-m4)
  out[4tx+2] = (m1+m2) + 4(m3+m4)
  out[4tx+3] = (m1-m2) + 8(m3-m4) + m5
with m staged PSUM->SBUF as bf16 by ScalarE and the combine on DVE
(tensor_tensor 2x + tensor_scalar 4x perf modes). Output ships bf16 and
is widened to fp32 on host.

MACs: 6 jx x 6 (cic,ky) x 512 -> 576 matmuls/core vs 768 for F(2,3).
The H-pad rows (global rows 0 and 65) are all-zero, so the one ky per
group that touches a pad row is trimmed to 31 rows (N=496), shaving
~1.3us off the PE-bound matmul stream (measured ~2-3us incl. knock-on
scheduling effects).
"""

import numpy as np
import ml_dtypes

B, T, CIN, COUT, H, W = 8, 4, 256, 256, 64, 64
KH, KW = 3, 3
NCORES = 8
CH = 2
NJX = 6             # winograd positions per tile
NTX = W // 4        # 16 tiles of 4 output cols per row
HP = H + 2          # 66 padded rows
YB_ROWS = 32        # output rows per psum tile -> N = 32*16 = 512
NYB = H // YB_ROWS  # 2

XROW = NJX * NTX    # 96 V values per padded row (stored plane-major)
NW = CH * NJX * CH * KH  # 72 weight tiles

_cache = {}
LAST_EXEC_TIME_NS = None
LAST_PROFILE = None


def _build():
    import concourse.mybir as mybir
    import concourse.tile as tile
    from concourse import bacc

    ALU = mybir.AluOpType

    nc = bacc.Bacc(
        "TRN2",
        target_bir_lowering=False,
        debug=False,
        enable_asserts=False,
        num_devices=NCORES,
    )
    x_d = nc.dram_tensor(
        "x", [T, CH, 128, HP * XROW], mybir.dt.bfloat16, kind="ExternalInput"
    ).ap()
    w_d = nc.dram_tensor(
        "w", [128, NW * 128], mybir.dt.bfloat16, kind="ExternalInput"
    ).ap()
    o_d = nc.dram_tensor(
        "out", [T, CH, 128, H * W], mybir.dt.bfloat16, kind="ExternalOutput"
    ).ap()

    ROW_BLOCKS = [(0, 34), (34, 50), (50, 66)]

    def widx(coc, jx, cic, ky):
        return ((coc * NJX + jx) * CH + cic) * KH + ky

    with tile.TileContext(nc) as tc:
        with (
            tc.tile_pool(name="persist", bufs=1) as persist,
            tc.tile_pool(name="xv", bufs=2) as xv_pool,
            tc.tile_pool(name="psum", bufs=8, space="PSUM") as psum_pool,
            tc.tile_pool(name="obuf", bufs=2) as out_pool,
        ):
            w_sb = persist.tile([128, NW * 128], mybir.dt.bfloat16, tag="w")

            # V tiles (host-transformed input), double-buffered across images
            x_sb = {}
            for t in range(T):
                for c in range(CH):
                    x_sb[(t, c)] = xv_pool.tile(
                        [128, HP * XROW],
                        mybir.dt.bfloat16,
                        name=f"x{t}{c}",
                        tag=f"x{c}",
                        bufs=2,
                    )

            # PE warmup bridge: DVE memset (GpSimd's queue launches ~5us
            # late) + tiny N=64 matmuls from ~7.3us to ~14us. This (a)
            # completes the pstate ramp (~5.3us of continuous PE busy)
            # BEFORE the first real chain, and (b) lets ~1.5MB of input
            # accumulate so the stream can then run at full clock without
            # the starvation gaps that would otherwise reset the ramp.
            warm_x = persist.tile([128, 192], mybir.dt.bfloat16, name="warm", tag="warm")
            warm_ps = psum_pool.tile(
                [128, 512], mybir.dt.float32, name="wps", tag="m", bufs=8
            )
            nc.vector.memset(warm_x[:], 0.0)
            for _ in range(160):
                nc.tensor.matmul(
                    warm_ps[:, :64], warm_x[:, :128], warm_x[:, 128:192],
                    start=True, stop=True,
                )

            # DMA order: first-needed slivers first; image-0 chunk 0 on the
            # sync ring, weights + chunk 1 on the scalar ring so the critical
            # tiles land in parallel
            # V is plane-major: [jx, HP, NTX] — matmul rhs slices are fully
            # contiguous runs, which the PE fetches at full rate (16-element
            # runs measured ~28% slower on HW)
            def xv3(t, c):
                return x_sb[(t, c)][:].rearrange(
                    "p (j h w) -> p j (h w)", j=NJX, w=NTX
                )

            def dma_x_block(t, c, blk, engine=None):
                r0, r1 = ROW_BLOCKS[blk]
                eng = engine or nc.sync
                eng.dma_start(
                    xv3(t, c)[:, :, r0 * NTX : r1 * NTX],
                    x_d[t, c, :].rearrange("p (j r) -> p j r", j=NJX)[
                        :, :, r0 * NTX : r1 * NTX
                    ],
                )

            def dma_x1_block(blk):
                dma_x_block(0, 1, blk, engine=nc.scalar)

            # image-0 weights + inputs are 5.6MB over two HWDGE rings —
            # balance ~2.8MB/ring and order by first-use time
            # ACT (scalar ring) issues only the first four DMAs — anything
            # later would interleave with the PSUM-drain copies and stall the
            # PE on bank frees; the sync ring carries the rest by need time
            # c1's block first on the scalar ring: at full-clock consumption
            # it is the binding arrival (chain k's 4th matmul); w[0:6] still
            # lands in time behind it
            dma_x1_block(0)
            nc.scalar.dma_start(w_sb[:, : 6 * 128], w_d[:, : 6 * 128])
            dma_x_block(0, 0, 0)
            nc.scalar.dma_start(w_sb[:, 6 * 128 : 18 * 128], w_d[:, 6 * 128 : 18 * 128])
            nc.scalar.dma_start(w_sb[:, 18 * 128 : 36 * 128], w_d[:, 18 * 128 : 36 * 128])
            dma_x_block(0, 0, 1)
            dma_x_block(0, 0, 2)
            dma_x_block(0, 1, 1, engine=nc.sync)
            dma_x_block(0, 1, 2, engine=nc.sync)
            nc.sync.dma_start(w_sb[:, 36 * 128 : 54 * 128], w_d[:, 36 * 128 : 54 * 128])
            nc.sync.dma_start(w_sb[:, 54 * 128 :], w_d[:, 54 * 128 :])
            for t in range(1, T):
                for blk in range(3):
                    for c in range(CH):
                        dma_x_block(t, c, blk)

            for t in range(T):
                v3 = {
                    c: x_sb[(t, c)][:].rearrange(
                        "p (j h w) -> p j h w", j=NJX, w=NTX
                    )
                    for c in range(CH)
                }
                # coc-outer order: image 0's first two groups reuse weight
                # tiles 0..35, pushing the coc1-weight deadline to ~24us
                group_order = [(coc, yb) for coc in range(CH) for yb in range(NYB)]
                for coc, yb in group_order:
                    y0 = yb * YB_ROWS
                    last = t == T - 1 and coc == CH - 1 and yb == NYB - 1
                    # final group: run the m0 chain last (split in two) so
                    # only a short N=256 chain gates the trailing output
                    jx_order = [1, 2, 3, 4, 5, 0] if last else range(NJX)
                    m = [None] * NJX
                    m0ab = []

                    def mm_chain(mp, jx, rr0, nrows):
                        # the H-pad rows are all-zero, so the ky touching
                        # one (ky=0 at global row 0, ky=2 at row 65) is
                        # trimmed to nrows-1 rows; a full-width ky runs
                        # first so start=True covers the whole psum tile
                        if rr0 == 0 and nrows == YB_ROWS:
                            ky_order, trim = (1, 0, 2), 0
                        elif rr0 + nrows + 2 == HP and nrows == YB_ROWS:
                            ky_order, trim = (0, 1, 2), 2
                        else:
                            ky_order, trim = (0, 1, 2), None
                        k = 0
                        for cic in range(CH):
                            for ky in ky_order:
                                idx = widx(coc, jx, cic, ky)
                                if ky == trim and ky == 0:
                                    rhs = v3[cic][
                                        :, jx, rr0 + 1 : rr0 + nrows, :
                                    ]
                                    out = mp[:, NTX : nrows * NTX]
                                elif ky == trim:
                                    rhs = v3[cic][
                                        :, jx,
                                        rr0 + ky : rr0 + ky + nrows - 1, :,
                                    ]
                                    out = mp[:, : (nrows - 1) * NTX]
                                else:
                                    rhs = v3[cic][
                                        :, jx, rr0 + ky : rr0 + ky + nrows, :
                                    ]
                                    out = mp[:]
                                nc.tensor.matmul(
                                    out,
                                    w_sb[:, idx * 128 : (idx + 1) * 128],
                                    rhs,
                                    start=(k == 0),
                                    stop=(k == CH * KH - 1),
                                )
                                k += 1

                    for jx in jx_order:
                        if last and jx == 0:
                            half = YB_ROWS // 2
                            for h in range(2):
                                mp = psum_pool.tile(
                                    [128, half * NTX],
                                    mybir.dt.float32,
                                    name=f"m0{h}",
                                    tag="m",
                                    bufs=8,
                                )
                                mm_chain(mp, 0, y0 + h * half, half)
                                m0ab.append(mp)
                            continue
                        mp = psum_pool.tile(
                            [128, YB_ROWS * NTX],
                            mybir.dt.float32,
                            name=f"m{jx}",
                            tag="m",
                            bufs=8,
                        )
                        mm_chain(mp, jx, y0, YB_ROWS)
                        m[jx] = mp

                    NEL = YB_ROWS * NTX
                    cst = [
                        out_pool.tile(
                            [128, NEL],
                            mybir.dt.bfloat16,
                            name=f"c{j}",
                            tag=f"c{j}",
                            bufs=2,
                        )
                        for j in range(NJX)
                    ]
                    for j in jx_order:
                        if last and j in (0, 5):
                            continue  # o0/o3 read m0/m5 straight from PSUM
                        nc.scalar.copy(cst[j][:], m[j][:])
                    s = out_pool.tile(
                        [128, NEL], mybir.dt.bfloat16, name="s", tag="s", bufs=2
                    )
                    dd = out_pool.tile(
                        [128, NEL], mybir.dt.bfloat16, name="dd", tag="dd", bufs=2
                    )
                    a = out_pool.tile(
                        [128, NEL], mybir.dt.bfloat16, name="a", tag="a", bufs=2
                    )
                    bb = out_pool.tile(
                        [128, NEL], mybir.dt.bfloat16, name="bb", tag="bb", bufs=2
                    )
                    sc = out_pool.tile(
                        [128, NEL], mybir.dt.bfloat16, name="sc", tag="sc", bufs=2
                    )
                    ob = out_pool.tile(
                        [128, 4 * NEL], mybir.dt.bfloat16, name="ob", tag="ob",
                        bufs=2,
                    )
                    o = [ob[:, j * NEL : (j + 1) * NEL] for j in range(4)]

                    def scaled_add(out, src, k, addend):
                        # (src * k) + addend via ts (4x) + tt (2x) — both
                        # faster DVE paths than the 1x scalar_tensor_tensor
                        nc.vector.tensor_scalar(
                            sc[:], src, k, 0.0, op0=ALU.mult, op1=ALU.add
                        )
                        nc.vector.tensor_add(out, addend, sc[:])

                    base = yb * 4 * NEL
                    if not last:
                        nc.vector.tensor_add(s[:], cst[1][:], cst[2][:])
                        nc.vector.tensor_sub(dd[:], cst[1][:], cst[2][:])
                        nc.vector.tensor_add(a[:], cst[3][:], cst[4][:])
                        nc.vector.tensor_sub(bb[:], cst[3][:], cst[4][:])
                        nc.vector.tensor_add(o[0], cst[0][:], s[:])
                        nc.vector.tensor_add(o[0], o[0], a[:])
                        scaled_add(o[1], bb[:], 2.0, dd[:])
                        scaled_add(o[2], a[:], 4.0, s[:])
                        scaled_add(o[3], bb[:], 8.0, dd[:])
                        nc.vector.tensor_add(o[3], o[3], cst[5][:])
                        nc.gpsimd.dma_start(
                            o_d[t, coc, :, base : base + 4 * NEL], ob[:]
                        )
                    else:
                        # final tile: the DVE sequence is ordered by data
                        # availability — s/dd/o1 unlock at cst2 (T-3.2us),
                        # the cst4-gated ops follow, the o2 pair runs on the
                        # otherwise-idle GpSimd, and o3/o0 read m5/m0
                        # straight from PSUM so only one short DVE op + a
                        # sliver DMA trails each of the last chain pieces
                        nc.vector.tensor_add(s[:], cst[1][:], cst[2][:])
                        nc.vector.tensor_sub(dd[:], cst[1][:], cst[2][:])
                        nc.vector.tensor_add(a[:], cst[3][:], cst[4][:])
                        nc.vector.tensor_sub(bb[:], cst[3][:], cst[4][:])
                        scaled_add(o[1], bb[:], 2.0, dd[:])
                        nc.scalar.dma_start(
                            o_d[t, coc, :, base + NEL : base + 2 * NEL],
                            ob[:, NEL : 2 * NEL],
                        )
                        o0p = cst[0]
                        nc.vector.tensor_add(o0p[:], s[:], a[:])
                        sc2 = cst[5]
                        nc.gpsimd.tensor_scalar(
                            sc2[:], a[:], 4.0, 0.0, op0=ALU.mult, op1=ALU.add
                        )
                        nc.gpsimd.tensor_add(o[2], s[:], sc2[:])
                        nc.sync.dma_start(
                            o_d[t, coc, :, base + 2 * NEL : base + 3 * NEL],
                            ob[:, 2 * NEL : 3 * NEL],
                        )
                        scaled_add(o[3], bb[:], 8.0, dd[:])
                        nc.vector.tensor_add(o[3], o[3], m[5][:])
                        nc.scalar.dma_start(
                            o_d[t, coc, :, base + 3 * NEL : base + 4 * NEL],
                            ob[:, 3 * NEL :],
                        )
                        # o0 halves read m0a/m0b straight from PSUM; the final
                        # slivers ship on the otherwise-idle scalar and sync
                        # rings to dodge the pool issue queue
                        HNEL = NEL // 2
                        nc.vector.tensor_add(
                            o[0][:, :HNEL], m0ab[0][:], o0p[:, :HNEL]
                        )
                        nc.scalar.dma_start(
                            o_d[t, coc, :, base : base + HNEL], o[0][:, :HNEL]
                        )
                        nc.vector.tensor_add(
                            o[0][:, HNEL:], m0ab[1][:], o0p[:, HNEL:]
                        )
                        nc.sync.dma_start(
                            o_d[t, coc, :, base + HNEL : base + NEL],
                            o[0][:, HNEL:],
                        )

    nc.compile()
    return nc


_GP = None


def _gprime():
    global _GP
    if _GP is None:
        G = np.array(
            [
                [1 / 4, 0, 0],
                [-1 / 6, -1 / 6, -1 / 6],
                [-1 / 6, 1 / 6, -1 / 6],
                [1 / 24, 1 / 12, 1 / 6],
                [1 / 24, -1 / 12, 1 / 6],
                [0, 0, 1],
            ],
            dtype=np.float64,
        )
        S = np.diag([4.0, -4.0, 4.0, 2.0, 2.0, 4.0])
        _GP = (S @ G).astype(np.float32)
    return _GP


# scaled B^T rows (the inverse scales are folded into the weights)
_BTS = np.array(
    [
        [1, 0, -1.25, 0, 0.25, 0],
        [0, 1, 1, -0.25, -0.25, 0],
        [0, 1, -1, -0.25, 0.25, 0],
        [0, -1, -0.5, 1, 0.5, 0],
        [0, 1, -0.5, -1, 0.5, 0],
        [0, 1, 0, -1.25, 0, 0.25],
    ],
    dtype=np.float32,
)


def _prep_inputs(inputs, ada_weight):
    bf16 = ml_dtypes.bfloat16
    Gp = _gprime()
    # column gather index: d[..., r, k, tx] = xpad[..., r, 4tx+k]
    cols = 4 * np.arange(NTX)[None, :] + np.arange(NJX)[:, None]  # [k, tx]
    in_maps = []
    for b in range(B):
        xb = inputs[b * T : (b + 1) * T].reshape(T, CH, 128, H, W).astype(bf16)
        xp = np.zeros((T, CH, 128, HP, W + 2), dtype=bf16)
        xp[..., 1 : H + 1, 1 : W + 1] = xb
        d = xp[..., cols].astype(np.float32)  # [T, CH, 128, HP, NJX(k), NTX]
        # winograd input transform V_j = BTS[j] . d  (host side), plane-major
        xd = np.einsum("jk,...rkx->...jrx", _BTS, d).astype(bf16)

        wb = ada_weight[b].astype(np.float32)  # [co, ci, ky, kx]
        g = np.einsum("jk,oiyk->joiy", Gp, wb)  # [jx, co, ci, ky]
        gt = g.reshape(NJX, CH, 128, CH, 128, KH)  # jx coc co cic ci ky
        wprep = gt.transpose(4, 1, 0, 3, 5, 2)  # ci coc jx cic ky co
        wprep = np.ascontiguousarray(wprep.astype(bf16)).reshape(128, NW * 128)
        in_maps.append({"x": xd.reshape(T, CH, 128, HP * XROW), "w": wprep})
    return in_maps


def _unpack_out(res):
    # [T, CH, 128, NYB, 4 j, 32 y, 16 tx] -> [T, C, H, W]
    arr = np.asarray(res, dtype=np.float32).reshape(T, CH, 128, NYB, 4, YB_ROWS, NTX)
    a = arr.transpose(0, 1, 2, 3, 5, 6, 4)  # t ch co yb y tx j
    return a.reshape(T, COUT, H, W)


def _setup_profiling():
    import sys
    import types

    try:
        from antenv.axon_hooks import get_axon_ntff_profile_hook  # noqa: F401

        return
    except ImportError:
        pass
    import antenv
    from trn_agent_boot.trn_boot import _ntff_profile_via_ctypes

    hook = _ntff_profile_via_ctypes("/opt/axon/libaxon_pjrt.so")
    m = types.ModuleType("antenv.axon_hooks")
    m.get_axon_ntff_profile_hook = lambda: hook
    m.set_axon_ntff_profile_hook = lambda h: None
    sys.modules["antenv.axon_hooks"] = m
    antenv.axon_hooks = m

    from concourse import bass_utils

    bass_utils.upload_artifacts = lambda tmpdir: f"file://{tmpdir}"


def kernel(inputs, ada_weight, profile=False, trace_kwargs=None):
    global LAST_EXEC_TIME_NS, LAST_PROFILE
    from concourse.bass_utils import run_bass_kernel_spmd

    if profile:
        _setup_profiling()
    if "nc" not in _cache:
        _cache["nc"] = _build()
    nc = _cache["nc"]

    in_maps = _prep_inputs(np.asarray(inputs), np.asarray(ada_weight))

    kwargs = {}
    if profile:
        kwargs["trace"] = True
        if trace_kwargs:
            kwargs.update(trace_kwargs)
    res = run_bass_kernel_spmd(nc, in_maps, core_ids=list(range(NCORES)), **kwargs)
    if profile:
        LAST_EXEC_TIME_NS = res.exec_time_ns
        LAST_PROFILE = res

    out = np.stack([_unpack_out(res.results[b]["out"]) for b in range(B)])
    return np.ascontiguousarray(out.reshape(B * T, COUT, H, W).astype(np.float32))



# revision 64
# speedup vs baseline: 1.0236x; 1.0126x over previous
"""1-D Winograd F(4,3) along W for the per-sample adaptive conv.

Host prep: pad, de-interleave into stride-4 phase planes, and apply the
(scaled) B^T input transform per group of 4 output columns:
  d = x_pad[4tx .. 4tx+5]
  V0 = d0 - 1.25 d2 + 0.25 d4          (= row0(B^T)/4)
  V1 = (d1+d2) - 0.25 (d3+d4)          (= -row1/4)
  V2 = (d1-d2) - 0.25 (d3-d4)          (= row2/4)
  V3 = (d3-d1) + 0.5 (d4-d2)           (= row3/2)
  V4 = -(d3-d1) + 0.5 (d4-d2)          (= row4/2)
  V5 = d1 - 1.25 d3 + 0.25 d5          (= row5/4)
plus the weight G-transform (inverse row scales folded in).

Device: the full contraction
  m_jx = sum_{cic,ky} Gw[jx][co,ci,ky] * V_jx[ci, y+ky, tx]   (TensorE)
and the A^T output transform
  out[4tx+0] = m0+m1+m2+m3+m4
  out[4tx+1] = (m1-m2) + 2(m3-m4)
  out[4tx+2] = (m1+m2) + 4(m3+m4)
  out[4tx+3] = (m1-m2) + 8(m3-m4) + m5
with m staged PSUM->SBUF as bf16 by ScalarE and the combine on DVE
(tensor_tensor 2x + tensor_scalar 4x perf modes). Output ships bf16 and
is widened to fp32 on host.

MACs: 6 jx x 6 (cic,ky) x 512 -> 576 matmuls/core vs 768 for F(2,3).
The H-pad rows (global rows 0 and 65) are all-zero, so the one ky per
group that touches a pad row is trimmed to 31 rows (N=496), shaving
~1.3us off the PE-bound matmul stream (measured ~2-3us incl. knock-on
scheduling effects).
"""

import numpy as np
import ml_dtypes

B, T, CIN, COUT, H, W = 8, 4, 256, 256, 64, 64
KH, KW = 3, 3
NCORES = 8
CH = 2
NJX = 6             # winograd positions per tile
NTX = W // 4        # 16 tiles of 4 output cols per row
HP = H + 2          # 66 padded rows
YB_ROWS = 32        # output rows per psum tile -> N = 32*16 = 512
NYB = H // YB_ROWS  # 2

XROW = NJX * NTX    # 96 V values per padded row (stored plane-major)
NW = CH * NJX * CH * KH  # 72 weight tiles

_cache = {}
LAST_EXEC_TIME_NS = None
LAST_PROFILE = None


def _build():
    import concourse.mybir as mybir
    import concourse.tile as tile
    from concourse import bacc

    ALU = mybir.AluOpType

    nc = bacc.Bacc(
        "TRN2",
        target_bir_lowering=False,
        debug=False,
        enable_asserts=False,
        num_devices=NCORES,
    )
    x_d = nc.dram_tensor(
        "x", [T, CH, 128, HP * XROW], mybir.dt.bfloat16, kind="ExternalInput"
    ).ap()
    w_d = nc.dram_tensor(
        "w", [128, NW * 128], mybir.dt.bfloat16, kind="ExternalInput"
    ).ap()
    o_d = nc.dram_tensor(
        "out", [T, CH, 128, H * W], mybir.dt.bfloat16, kind="ExternalOutput"
    ).ap()

    ROW_BLOCKS = [(0, 34), (34, 50), (50, 66)]

    def widx(coc, jx, cic, ky):
        return ((coc * NJX + jx) * CH + cic) * KH + ky

    with tile.TileContext(nc) as tc:
        with (
            tc.tile_pool(name="persist", bufs=1) as persist,
            tc.tile_pool(name="xv", bufs=2) as xv_pool,
            tc.tile_pool(name="psum", bufs=8, space="PSUM") as psum_pool,
            tc.tile_pool(name="obuf", bufs=2) as out_pool,
        ):
            w_sb = persist.tile([128, NW * 128], mybir.dt.bfloat16, tag="w")

            # V tiles (host-transformed input), double-buffered across images
            x_sb = {}
            for t in range(T):
                for c in range(CH):
                    x_sb[(t, c)] = xv_pool.tile(
                        [128, HP * XROW],
                        mybir.dt.bfloat16,
                        name=f"x{t}{c}",
                        tag=f"x{c}",
                        bufs=2,
                    )

            # PE warmup bridge: DVE memset (GpSimd's queue launches ~5us
            # late) + tiny N=64 matmuls from ~7.3us to ~14us. This (a)
            # completes the pstate ramp (~5.3us of continuous PE busy)
            # BEFORE the first real chain, and (b) lets ~1.5MB of input
            # accumulate so the stream can then run at full clock without
            # the starvation gaps that would otherwise reset the ramp.
            warm_x = persist.tile([128, 192], mybir.dt.bfloat16, name="warm", tag="warm")
            warm_ps = psum_pool.tile(
                [128, 512], mybir.dt.float32, name="wps", tag="m", bufs=8
            )
            nc.vector.memset(warm_x[:], 0.0)
            for _ in range(215):
                nc.tensor.matmul(
                    warm_ps[:, :64], warm_x[:, :128], warm_x[:, 128:192],
                    start=True, stop=True,
                )

            # DMA order: first-needed slivers first; image-0 chunk 0 on the
            # sync ring, weights + chunk 1 on the scalar ring so the critical
            # tiles land in parallel
            # V is plane-major: [jx, HP, NTX] — matmul rhs slices are fully
            # contiguous runs, which the PE fetches at full rate (16-element
            # runs measured ~28% slower on HW)
            def xv3(t, c):
                return x_sb[(t, c)][:].rearrange(
                    "p (j h w) -> p j (h w)", j=NJX, w=NTX
                )

            def dma_x_block(t, c, blk, engine=None):
                r0, r1 = ROW_BLOCKS[blk]
                eng = engine or nc.sync
                eng.dma_start(
                    xv3(t, c)[:, :, r0 * NTX : r1 * NTX],
                    x_d[t, c, :].rearrange("p (j r) -> p j r", j=NJX)[
                        :, :, r0 * NTX : r1 * NTX
                    ],
                )

            def dma_x1_block(blk):
                dma_x_block(0, 1, blk, engine=nc.scalar)

            # image-0 weights + inputs are 5.6MB over two HWDGE rings —
            # balance ~2.8MB/ring and order by first-use time
            # ACT (scalar ring) issues only the first four DMAs — anything
            # later would interleave with the PSUM-drain copies and stall the
            # PE on bank frees; the sync ring carries the rest by need time
            # c1's block first on the scalar ring: at full-clock consumption
            # it is the binding arrival (chain k's 4th matmul); w[0:6] still
            # lands in time behind it
            dma_x1_block(0)
            nc.scalar.dma_start(w_sb[:, : 6 * 128], w_d[:, : 6 * 128])
            dma_x_block(0, 0, 0)
            nc.scalar.dma_start(w_sb[:, 6 * 128 : 18 * 128], w_d[:, 6 * 128 : 18 * 128])
            nc.scalar.dma_start(w_sb[:, 18 * 128 : 36 * 128], w_d[:, 18 * 128 : 36 * 128])
            dma_x_block(0, 0, 1)
            dma_x_block(0, 0, 2)
            dma_x_block(0, 1, 1, engine=nc.sync)
            dma_x_block(0, 1, 2, engine=nc.sync)
            nc.sync.dma_start(w_sb[:, 36 * 128 : 54 * 128], w_d[:, 36 * 128 : 54 * 128])
            nc.sync.dma_start(w_sb[:, 54 * 128 :], w_d[:, 54 * 128 :])
            for t in range(1, T):
                for blk in range(3):
                    for c in range(CH):
                        dma_x_block(t, c, blk)

            for t in range(T):
                v3 = {
                    c: x_sb[(t, c)][:].rearrange(
                        "p (j h w) -> p j h w", j=NJX, w=NTX
                    )
                    for c in range(CH)
                }
                # coc-outer order: image 0's first two groups reuse weight
                # tiles 0..35, pushing the coc1-weight deadline to ~24us
                group_order = [(coc, yb) for coc in range(CH) for yb in range(NYB)]
                for coc, yb in group_order:
                    y0 = yb * YB_ROWS
                    last = t == T - 1 and coc == CH - 1 and yb == NYB - 1
                    # final group: run the m0 chain last (split in two) so
                    # only a short N=256 chain gates the trailing output
                    jx_order = [1, 2, 3, 4, 5, 0] if last else range(NJX)
                    m = [None] * NJX
                    m0ab = []

                    def mm_chain(mp, jx, rr0, nrows):
                        # the H-pad rows are all-zero, so the ky touching
                        # one (ky=0 at global row 0, ky=2 at row 65) is
                        # trimmed to nrows-1 rows; a full-width ky runs
                        # first so start=True covers the whole psum tile
                        if rr0 == 0 and nrows == YB_ROWS:
                            ky_order, trim = (1, 0, 2), 0
                        elif rr0 + nrows + 2 == HP and nrows == YB_ROWS:
                            ky_order, trim = (0, 1, 2), 2
                        else:
                            ky_order, trim = (0, 1, 2), None
                        k = 0
                        for cic in range(CH):
                            for ky in ky_order:
                                idx = widx(coc, jx, cic, ky)
                                if ky == trim and ky == 0:
                                    rhs = v3[cic][
                                        :, jx, rr0 + 1 : rr0 + nrows, :
                                    ]
                                    out = mp[:, NTX : nrows * NTX]
                                elif ky == trim:
                                    rhs = v3[cic][
                                        :, jx,
                                        rr0 + ky : rr0 + ky + nrows - 1, :,
                                    ]
                                    out = mp[:, : (nrows - 1) * NTX]
                                else:
                                    rhs = v3[cic][
                                        :, jx, rr0 + ky : rr0 + ky + nrows, :
                                    ]
                                    out = mp[:]
                                nc.tensor.matmul(
                                    out,
                                    w_sb[:, idx * 128 : (idx + 1) * 128],
                                    rhs,
                                    start=(k == 0),
                                    stop=(k == CH * KH - 1),
                                )
                                k += 1

                    for jx in jx_order:
                        if last and jx == 0:
                            half = YB_ROWS // 2
                            for h in range(2):
                                mp = psum_pool.tile(
                                    [128, half * NTX],
                                    mybir.dt.float32,
                                    name=f"m0{h}",
                                    tag="m",
                                    bufs=8,
                                )
                                mm_chain(mp, 0, y0 + h * half, half)
                                m0ab.append(mp)
                            continue
                        mp = psum_pool.tile(
                            [128, YB_ROWS * NTX],
                            mybir.dt.float32,
                            name=f"m{jx}",
                            tag="m",
                            bufs=8,
                        )
                        mm_chain(mp, jx, y0, YB_ROWS)
                        m[jx] = mp

                    NEL = YB_ROWS * NTX
                    cst = [
                        out_pool.tile(
                            [128, NEL],
                            mybir.dt.bfloat16,
                            name=f"c{j}",
                            tag=f"c{j}",
                            bufs=2,
                        )
                        for j in range(NJX)
                    ]
                    for j in jx_order:
                        if last and j in (0, 5):
                            continue  # o0/o3 read m0/m5 straight from PSUM
                        nc.scalar.copy(cst[j][:], m[j][:])
                    s = out_pool.tile(
                        [128, NEL], mybir.dt.bfloat16, name="s", tag="s", bufs=2
                    )
                    dd = out_pool.tile(
                        [128, NEL], mybir.dt.bfloat16, name="dd", tag="dd", bufs=2
                    )
                    a = out_pool.tile(
                        [128, NEL], mybir.dt.bfloat16, name="a", tag="a", bufs=2
                    )
                    bb = out_pool.tile(
                        [128, NEL], mybir.dt.bfloat16, name="bb", tag="bb", bufs=2
                    )
                    sc = out_pool.tile(
                        [128, NEL], mybir.dt.bfloat16, name="sc", tag="sc", bufs=2
                    )
                    ob = out_pool.tile(
                        [128, 4 * NEL], mybir.dt.bfloat16, name="ob", tag="ob",
                        bufs=2,
                    )
                    o = [ob[:, j * NEL : (j + 1) * NEL] for j in range(4)]

                    def scaled_add(out, src, k, addend):
                        # (src * k) + addend via ts (4x) + tt (2x) — both
                        # faster DVE paths than the 1x scalar_tensor_tensor
                        nc.vector.tensor_scalar(
                            sc[:], src, k, 0.0, op0=ALU.mult, op1=ALU.add
                        )
                        nc.vector.tensor_add(out, addend, sc[:])

                    base = yb * 4 * NEL
                    if not last:
                        nc.vector.tensor_add(s[:], cst[1][:], cst[2][:])
                        nc.vector.tensor_sub(dd[:], cst[1][:], cst[2][:])
                        nc.vector.tensor_add(a[:], cst[3][:], cst[4][:])
                        nc.vector.tensor_sub(bb[:], cst[3][:], cst[4][:])
                        nc.vector.tensor_add(o[0], cst[0][:], s[:])
                        nc.vector.tensor_add(o[0], o[0], a[:])
                        scaled_add(o[1], bb[:], 2.0, dd[:])
                        scaled_add(o[2], a[:], 4.0, s[:])
                        scaled_add(o[3], bb[:], 8.0, dd[:])
                        nc.vector.tensor_add(o[3], o[3], cst[5][:])
                        nc.gpsimd.dma_start(
                            o_d[t, coc, :, base : base + 4 * NEL], ob[:]
                        )
                    else:
                        # final tile: the DVE sequence is ordered by data
                        # availability — s/dd/o1 unlock at cst2 (T-3.2us),
                        # the cst4-gated ops follow, the o2 pair runs on the
                        # otherwise-idle GpSimd, and o3/o0 read m5/m0
                        # straight from PSUM so only one short DVE op + a
                        # sliver DMA trails each of the last chain pieces
                        nc.vector.tensor_add(s[:], cst[1][:], cst[2][:])
                        nc.vector.tensor_sub(dd[:], cst[1][:], cst[2][:])
                        nc.vector.tensor_add(a[:], cst[3][:], cst[4][:])
                        nc.vector.tensor_sub(bb[:], cst[3][:], cst[4][:])
                        scaled_add(o[1], bb[:], 2.0, dd[:])
                        nc.scalar.dma_start(
                            o_d[t, coc, :, base + NEL : base + 2 * NEL],
                            ob[:, NEL : 2 * NEL],
                        )
                        o0p = cst[0]
                        nc.vector.tensor_add(o0p[:], s[:], a[:])
                        sc2 = cst[5]
                        nc.gpsimd.tensor_scalar(
                            sc2[:], a[:], 4.0, 0.0, op0=ALU.mult, op1=ALU.add
                        )
                        nc.gpsimd.tensor_add(o[2], s[:], sc2[:])
                        nc.sync.dma_start(
                            o_d[t, coc, :, base + 2 * NEL : base + 3 * NEL],
                            ob[:, 2 * NEL : 3 * NEL],
                        )
                        scaled_add(o[3], bb[:], 8.0, dd[:])
                        nc.vector.tensor_add(o[3], o[3], m[5][:])
                        nc.scalar.dma_start(
                            o_d[t, coc, :, base + 3 * NEL : base + 4 * NEL],
                            ob[:, 3 * NEL :],
                        )
                        # o0 halves read m0a/m0b straight from PSUM; the final
                        # slivers ship on the otherwise-idle scalar and sync
                        # rings to dodge the pool issue queue
                        HNEL = NEL // 2
                        nc.vector.tensor_add(
                            o[0][:, :HNEL], m0ab[0][:], o0p[:, :HNEL]
                        )
                        nc.scalar.dma_start(
                            o_d[t, coc, :, base : base + HNEL], o[0][:, :HNEL]
                        )
                        nc.vector.tensor_add(
                            o[0][:, HNEL:], m0ab[1][:], o0p[:, HNEL:]
                        )
                        nc.sync.dma_start(
                            o_d[t, coc, :, base + HNEL : base + NEL],
                            o[0][:, HNEL:],
                        )

    nc.compile()
    return nc


_GP = None


def _gprime():
    global _GP
    if _GP is None:
        G = np.array(
            [
                [1 / 4, 0, 0],
                [-1 / 6, -1 / 6, -1 / 6],
                [-1 / 6, 1 / 6, -1 / 6],
                [1 / 24, 1 / 12, 1 / 6],
                [1 / 24, -1 / 12, 1 / 6],
                [0, 0, 1],
            ],
            dtype=np.float64,
        )
        S = np.diag([4.0, -4.0, 4.0, 2.0, 2.0, 4.0])
        _GP = (S @ G).astype(np.float32)
    return _GP


# scaled B^T rows (the inverse scales are folded into the weights)
_BTS = np.array(
    [
        [1, 0, -1.25, 0, 0.25, 0],
        [0, 1, 1, -0.25, -0.25, 0],
        [0, 1, -1, -0.25, 0.25, 0],
        [0, -1, -0.5, 1, 0.5, 0],
        [0, 1, -0.5, -1, 0.5, 0],
        [0, 1, 0, -1.25, 0, 0.25],
    ],
    dtype=np.float32,
)


def _prep_inputs(inputs, ada_weight):
    bf16 = ml_dtypes.bfloat16
    Gp = _gprime()
    # column gather index: d[..., r, k, tx] = xpad[..., r, 4tx+k]
    cols = 4 * np.arange(NTX)[None, :] + np.arange(NJX)[:, None]  # [k, tx]
    in_maps = []
    for b in range(B):
        xb = inputs[b * T : (b + 1) * T].reshape(T, CH, 128, H, W).astype(bf16)
        xp = np.zeros((T, CH, 128, HP, W + 2), dtype=bf16)
        xp[..., 1 : H + 1, 1 : W + 1] = xb
        d = xp[..., cols].astype(np.float32)  # [T, CH, 128, HP, NJX(k), NTX]
        # winograd input transform V_j = BTS[j] . d  (host side), plane-major
        xd = np.einsum("jk,...rkx->...jrx", _BTS, d).astype(bf16)

        wb = ada_weight[b].astype(np.float32)  # [co, ci, ky, kx]
        g = np.einsum("jk,oiyk->joiy", Gp, wb)  # [jx, co, ci, ky]
        gt = g.reshape(NJX, CH, 128, CH, 128, KH)  # jx coc co cic ci ky
        wprep = gt.transpose(4, 1, 0, 3, 5, 2)  # ci coc jx cic ky co
        wprep = np.ascontiguousarray(wprep.astype(bf16)).reshape(128, NW * 128)
        in_maps.append({"x": xd.reshape(T, CH, 128, HP * XROW), "w": wprep})
    return in_maps


def _unpack_out(res):
    # [T, CH, 128, NYB, 4 j, 32 y, 16 tx] -> [T, C, H, W]
    arr = np.asarray(res, dtype=np.float32).reshape(T, CH, 128, NYB, 4, YB_ROWS, NTX)
    a = arr.transpose(0, 1, 2, 3, 5, 6, 4)  # t ch co yb y tx j
    return a.reshape(T, COUT, H, W)


def _setup_profiling():
    import sys
    import types

    try:
        from antenv.axon_hooks import get_axon_ntff_profile_hook  # noqa: F401

        return
    except ImportError:
        pass
    import antenv
    from trn_agent_boot.trn_boot import _ntff_profile_via_ctypes

    hook = _ntff_profile_via_ctypes("/opt/axon/libaxon_pjrt.so")
    m = types.ModuleType("antenv.axon_hooks")
    m.get_axon_ntff_profile_hook = lambda: hook
    m.set_axon_ntff_profile_hook = lambda h: None
    sys.modules["antenv.axon_hooks"] = m
    antenv.axon_hooks = m

    from concourse import bass_utils

    bass_utils.upload_artifacts = lambda tmpdir: f"file://{tmpdir}"


def kernel(inputs, ada_weight, profile=False, trace_kwargs=None):
    global LAST_EXEC_TIME_NS, LAST_PROFILE
    from concourse.bass_utils import run_bass_kernel_spmd

    if profile:
        _setup_profiling()
    if "nc" not in _cache:
        _cache["nc"] = _build()
    nc = _cache["nc"]

    in_maps = _prep_inputs(np.asarray(inputs), np.asarray(ada_weight))

    kwargs = {}
    if profile:
        kwargs["trace"] = True
        if trace_kwargs:
            kwargs.update(trace_kwargs)
    res = run_bass_kernel_spmd(nc, in_maps, core_ids=list(range(NCORES)), **kwargs)
    if profile:
        LAST_EXEC_TIME_NS = res.exec_time_ns
        LAST_PROFILE = res

    out = np.stack([_unpack_out(res.results[b]["out"]) for b in range(B)])
    return np.ascontiguousarray(out.reshape(B * T, COUT, H, W).astype(np.float32))

